# revision 27
# baseline (speedup 1.0000x reference)
"""CayleyConv forward on 8 Trainium2 NeuronCores (Bass/Tile), bf16 I/O.

Problem: x [16,16,128,128,3,3] f32, g [16,16,8,3] f32
         -> out [16,16,130,130,3,3] f32.

Math (faithful to the reference's sequential-overwrite semantics):
  - Interior (rows/cols 1..128) is a 5-tap stencil on xl = x[:, -1] only:
    a [45, 144] contraction per padded position (taps x 9ch -> 16ch x 9).
  - The 1-pixel border keeps full accumulation over all input channels:
    per border line a K=432 contraction.

Sharding: data parallel over batch, 2 images/core. Weights built on host.

Device-side design (v2):
  - All device I/O in bf16 (tolerance 2e-2; bf16 error ~4e-3): halves HBM
    traffic vs f32 -> ~14.4 MB/core, ~40us at the 358 GB/s HBM/NC limit.
  - Position-pair stacking: rhs [90, n] holds the 45 tap-rows for position
    column j (rows 0:45) and j+HALF (rows 45:90). Three matmul streams with
    block-diagonal weights produce 288 output rows per column-pair in 3
    matmuls (M=128/128/32) -> 1.5 streamed cols/position instead of 2.
  - Stream 3 (M=32) accumulates 4 blocks into one [128,512] PSUM bank via
    zero-padded weight variants (block a lands on partitions 32a..32a+31),
    so PSUM evacuation is one [128,512] copy per span.
  - PSUM evacuation (the FD-cost-bound stage: cost = free-dim elems per
    instruction regardless of partitions) alternates Vector/Scalar engines,
    always 128 partitions x 512 cols, casting f32->bf16 during the copy.
  - Stores are [128, n] bf16 tiles -> even descriptor spread over SDMA
    engines; interior loads are DMA-replicated tap reads (grouped taps
    {0,+1} and {+S-1,+S,+S+1} as 2 APs per position-block).
"""

import numpy as np
import ml_dtypes

BF16 = ml_dtypes.bfloat16

# ---------------- problem constants (hardcoded per contract) ----------------
B, CIN, O, H, W = 16, 16, 16, 128, 128
NCORES = 8
BLOC = B // NCORES          # 2 images per core
S = W + 1                   # padded row stride (zero pad col at w=W)
PH = H + 1                  # padded rows per image (zero pad row at h=H)
IMG = PH * S                # 16641 padded positions per image
NPOS = BLOC * IMG           # 33282
HALF = NPOS // 2            # 16641 positions per stacked block
TILE_N = 512
NBLK = 33                   # ceil(HALF / TILE_N) pair-column blocks
NCP = NBLK * TILE_N         # 16896 padded pair-columns
XIN_COLS = 2 * NCP + 132    # 33924: tail pad so +HALF+130 reads stay in-bounds
SPAN_BLKS = 4               # blocks per span (store granularity)
NSPAN = -(-NBLK // SPAN_BLKS)   # 9 spans
D3_COLS = NSPAN * TILE_N    # 4608
TAP_OFFS = (0, 1, S - 1, S, S + 1)
N_TB = BLOC * (W + 2)       # 260 (top/bottom line positions)
N_LR = BLOC * H             # 256 (left/right line positions)
BOR_OFF = (0, N_TB, 2 * N_TB, 2 * N_TB + N_LR)
BOR_COLS = 2 * N_TB + 2 * N_LR  # 1032
KBOR = CIN * 27             # 432 contraction rows for border
IGRPS = 4
KG = KBOR // IGRPS          # 108
EPS = 1e-7

_CACHE = {}


# ---------------- host-side math (tiny) ----------------
def _build_G(g):
    # g: [O, I, 8, 3] f32 -> Cayley matrices G [O, I, 3, 3, 3, 3]
    idx = np.array([[0, 1, 2], [3, 4, 5], [6, 7, 4]])
    gk = g[:, :, idx, :]
    a_, b_, c_ = gk[..., 0], gk[..., 1], gk[..., 2]
    z = np.zeros_like(a_)
    A = np.stack([
        np.stack([z, a_, b_], -1),
        np.stack([-a_, z, c_], -1),
        np.stack([-b_, -c_, z], -1),
    ], -2)
    I3 = np.eye(3, dtype=g.dtype)
    Xm = A.copy(); Xm[:, :, 1, 1] = I3
    Idm = (I3 - A); Idm[:, :, 1, 1] = I3
    bm = Idm
    b00 = bm[..., 0, 0]; b01 = bm[..., 0, 1]; b02 = bm[..., 0, 2]
    b10 = bm[..., 1, 0]; b11 = bm[..., 1, 1]; b12 = bm[..., 1, 2]
    b20 = bm[..., 2, 0]; b21 = bm[..., 2, 1]; b22 = bm[..., 2, 2]
    det = (b00 * (b11 * b22 - b12 * b21)
           - b01 * (b10 * b22 - b12 * b20)
           + b02 * (b10 * b21 - b11 * b20))
    cof = np.stack([
        np.stack([b11 * b22 - b12 * b21, b02 * b21 - b01 * b22, b01 * b12 - b02 * b11], -1),
        np.stack([b12 * b20 - b10 * b22, b00 * b22 - b02 * b20, b02 * b10 - b00 * b12], -1),
        np.stack([b10 * b21 - b11 * b20, b01 * b20 - b00 * b21, b00 * b11 - b01 * b10], -1),
    ], -2)
    inv = cof / (det + EPS)[..., None, None]
    return (inv @ (I3 + Xm)).astype(np.float32)


def _weights(g):
    """Returns (w1 [90,128], w2 [90,128], w3 [90,512], w_bor [108,2304]) f32."""
    # (packed into one [108, 3072] DRAM tensor for a single low-descriptor DMA)
    G = _build_G(g)                                     # [O, I, 3, 3, 3, 3]
    K9 = np.einsum('oiabpq,oiabts->oiabptqs', G, G).reshape(O, CIN, 3, 3, 9, 9)
    K9 = K9.astype(np.float32)

    # interior: taps in DMA-offset order [0, +1, +S-1, +S, +S+1]
    K_int = np.empty((5, O, 9, 9), np.float32)
    K_int[0] = np.eye(9, dtype=np.float32)
    K_int[1] = K9[:, CIN - 1, 1, 0]
    K_int[2] = K9[:, CIN - 1, 0, 2]
    K_int[3] = K9[:, CIN - 1, 0, 1]
    K_int[4] = K9[:, CIN - 1, 0, 0]
    # w_int[t*9+q, o*9+p] = K_int[t, o, p, q]
    w_int = np.ascontiguousarray(K_int.transpose(0, 3, 1, 2).reshape(45, O * 9))

    # block-diagonal packing over the two stacked position blocks:
    #  stream 1 [90,128]: block1 output rows 0..127
    #  stream 2 [90,128]: cols 0:16 = block1 rows 128..143 (top half),
    #                     cols 16:128 = block2 rows 0..111 (bottom half)
    #  stream 3: 4 zero-padded [90,128] variants; variant a holds block2
    #            rows 112..143 at output partitions 32a..32a+31 (accumulated
    #            over the 4 blocks of a span in one PSUM bank)
    w1 = np.zeros((90, 128), np.float32)
    w1[0:45] = w_int[:, 0:128]
    w2 = np.zeros((90, 128), np.float32)
    w2[0:45, 0:16] = w_int[:, 128:144]
    w2[45:90, 16:128] = w_int[:, 0:112]
    w3 = np.zeros((90, 4 * 128), np.float32)
    for a in range(4):
        w3[45:90, a * 128 + 32 * a: a * 128 + 32 * a + 32] = w_int[:, 112:144]

    # border lines: top (a=0, taps b), bottom (a=2, taps b),
    #               left (b=0, taps a), right (b=2, taps a)
    w_bor = np.empty((KG, 4 * IGRPS * O * 9), np.float32)
    sels = [K9[:, :, 0, :], K9[:, :, 2, :], K9[:, :, :, 0], K9[:, :, :, 2]]
    for L, KL in enumerate(sels):                       # KL [O, I, 3, 9p, 9q]
        WL = KL.transpose(1, 2, 4, 0, 3).reshape(KBOR, O * 9)  # rows (i,t,q)
        for j in range(IGRPS):
            w_bor[:, (L * IGRPS + j) * 144:(L * IGRPS + j + 1) * 144] = \
                WL[j * KG:(j + 1) * KG]
    return w1, w2, w3, w_bor


def _prep_xin_int(x, c):
    xsl = x[BLOC * c:BLOC * c + BLOC, CIN - 1]          # [2, H, W, 3, 3]
    xl9 = xsl.reshape(BLOC, H, W, 9).transpose(3, 0, 1, 2)
    tmp = np.zeros((9, BLOC, PH, S), np.float32)
    tmp[:, :, :H, :W] = xl9
    out = np.zeros((9, XIN_COLS), np.float32)
    out[:, :NPOS] = tmp.reshape(9, NPOS)
    return out


def _prep_xin_bor(x, c):
    x9 = x[BLOC * c:BLOC * c + BLOC].reshape(BLOC, CIN, H, W, 9)
    bor = np.zeros((KBOR, BOR_COLS), np.float32)
    for li, h_in in ((0, 0), (1, H - 1)):               # top, bottom
        rT = x9[:, :, h_in].transpose(1, 3, 0, 2)       # [I, 9, 2, W]
        blk = np.zeros((CIN, 3, 9, BLOC, W + 2), np.float32)
        for b in range(3):
            blk[:, b, :, :, b:b + W] = rT
        bor[:, BOR_OFF[li]:BOR_OFF[li] + N_TB] = blk.reshape(KBOR, N_TB)
    for li, w_in in ((2, 0), (3, W - 1)):               # left, right
        cT = x9[:, :, :, w_in].transpose(1, 3, 0, 2)    # [I, 9, 2, H]
        blk = np.zeros((CIN, 3, 9, BLOC, H), np.float32)
        for a in range(3):
            h_lo = max(0, 1 - a)
            h_hi = min(H - 1, H - a)
            blk[:, a, :, :, h_lo + a - 1:h_hi + a] = cT[:, :, :, h_lo:h_hi + 1]
        bor[:, BOR_OFF[li]:BOR_OFF[li] + N_LR] = blk.reshape(KBOR, N_LR)
    return bor


def _decode_full(res):
    """Rebuild full [144, NPOS] f32 from d12/d3 device outputs."""
    # d12 [128, 2*NCP]: block b cols [1024b,1024b+512) = stream1 (d1),
    # [1024b+512,1024b+1024) = stream2 (d2)
    d12 = np.asarray(res["d12"]).astype(np.float32).reshape(128, NBLK, 2, TILE_N)
    d1 = d12[:, :, 0, :].reshape(128, NCP)
    d2 = d12[:, :, 1, :].reshape(128, NCP)
    d3 = np.asarray(res["d3"]).astype(np.float32)       # [128, D3_COLS]
    full = np.empty((144, NPOS), np.float32)
    full[0:128, 0:HALF] = d1[:, :HALF]
    full[128:144, 0:HALF] = d2[0:16, :HALF]
    full[0:112, HALF:] = d2[16:128, :HALF]
    # d3[32a+r, g*512+u] = block2 row 112+r at pair-col (4g+a)*512+u
    d3r = d3.reshape(4, 32, NSPAN, TILE_N)              # [a, r, g, u]
    blk2 = d3r.transpose(1, 2, 0, 3).reshape(32, NSPAN * 4 * TILE_N)
    full[112:144, HALF:] = blk2[:, :HALF]
    return full


def _assemble(results):
    out = np.empty((B, O, H + 2, W + 2, 3, 3), np.float32)
    for c in range(NCORES):
        ms = slice(BLOC * c, BLOC * c + BLOC)
        full = _decode_full(results[c])
        t = full.reshape(O, 9, BLOC, PH, S)[:, :, :, :H, :W]
        out[ms, :, 1:H + 1, 1:W + 1] = \
            t.transpose(2, 0, 3, 4, 1).reshape(BLOC, O, H, W, 3, 3)
        ob = np.asarray(results[c]["out_bor"]).astype(np.float32)
        ob = ob.reshape(O, 9, BOR_COLS)
        top = ob[:, :, 0:N_TB].reshape(O, 9, BLOC, W + 2)
        out[ms, :, 0, :] = top.transpose(2, 0, 3, 1).reshape(BLOC, O, W + 2, 3, 3)
        bot = ob[:, :, N_TB:2 * N_TB].reshape(O, 9, BLOC, W + 2)
        out[ms, :, H + 1, :] = bot.transpose(2, 0, 3, 1).reshape(BLOC, O, W + 2, 3, 3)
        lef = ob[:, :, BOR_OFF[2]:BOR_OFF[2] + N_LR].reshape(O, 9, BLOC, H)
        out[ms, :, 1:H + 1, 0] = lef.transpose(2, 0, 3, 1).reshape(BLOC, O, H, 3, 3)
        rig = ob[:, :, BOR_OFF[3]:BOR_OFF[3] + N_LR].reshape(O, 9, BLOC, H)
        out[ms, :, 1:H + 1, W + 1] = rig.transpose(2, 0, 3, 1).reshape(BLOC, O, H, 3, 3)
    return out


# ---------------- numpy simulation of the device program ----------------
def _sim_core(inm, fp=np.float32):
    """Simulate the device kernel for one core's input map (for validation)."""
    xin9 = np.asarray(inm["xin9"]).astype(np.float32)
    w1 = np.asarray(inm["w1"]).astype(np.float32)
    w2 = np.asarray(inm["w2"]).astype(np.float32)
    w3 = np.asarray(inm["w3"]).astype(np.float32)
    w_bor = np.asarray(inm["w_bor"]).astype(np.float32)
    xin_bor = np.asarray(inm["xin_bor"]).astype(np.float32)

    def taps(c0, n):
        r = np.empty((45, n), np.float32)
        for t, off in enumerate(TAP_OFFS):
            r[t * 9:(t + 1) * 9] = xin9[:, c0 + off:c0 + off + n]
        return r

    d12 = np.zeros((128, NBLK, 2, TILE_N), fp)
    d3 = np.zeros((128, D3_COLS), fp)
    for b0 in range(0, NBLK, SPAN_BLKS):
        nb = min(SPAN_BLKS, NBLK - b0)
        c0 = b0 * TILE_N
        ncols = nb * TILE_N
        rhs = np.concatenate([taps(c0, ncols), taps(c0 + HALF, ncols)], 0)
        p3 = np.zeros((128, TILE_N), np.float32)
        for a in range(nb):
            j0 = a * TILE_N
            rb = rhs[:, j0:j0 + TILE_N]
            d12[:, b0 + a, 0] = (w1.T @ rb).astype(fp)
            d12[:, b0 + a, 1] = (w2.T @ rb).astype(fp)
            p3 += w3[:, a * 128:(a + 1) * 128].T @ rb
        d3[:, (b0 // SPAN_BLKS) * TILE_N:(b0 // SPAN_BLKS + 1) * TILE_N] = \
            p3.astype(fp)
    d12 = d12.reshape(128, 2 * NCP)

    ret = {"d12": d12, "d3": d3}
    out_bor = np.zeros((144, BOR_COLS), fp)
    for L in range(4):
        ncol = N_TB if L < 2 else N_LR
        coff = BOR_OFF[L]
        for half in range(2):
            acc = np.zeros((72, ncol), np.float32)
            for j in range(IGRPS):
                wc = (L * IGRPS + j) * 144 + half * 72
                acc += w_bor[:, wc:wc + 72].T @ \
                    xin_bor[j * KG:(j + 1) * KG, coff:coff + ncol]
            out_bor[half * 72:half * 72 + 72, coff:coff + ncol] = acc.astype(fp)
    ret["out_bor"] = out_bor
    return ret


# ---------------- device kernel ----------------
# wcat layout: [108, 3072] bf16. rows 0:90 cols 0:128 = w1, 128:256 = w2,
# 256:768 = w3 (4 variants); rows 0:108 cols 768:3072 = w_bor [108, 2304].
WB_OFF = 768


def _build_module():
    if "nc" in _CACHE:
        return _CACHE["nc"]
    import concourse.bass as bass
    import concourse.mybir as mybir
    import concourse.tile as tile
    from concourse import bacc

    f32 = mybir.dt.float32
    bf16 = mybir.dt.bfloat16
    nc = bacc.Bacc(None, target_bir_lowering=False)

    xin18 = nc.dram_tensor("xin18", [18, XCOLS2], bf16, kind="ExternalInput")
    bor4 = nc.dram_tensor("bor4", [KG, 4 * BOR_COLS], bf16, kind="ExternalInput")
    wcat = nc.dram_tensor("wcat", [KG, 3072], bf16, kind="ExternalInput")
    d12 = nc.dram_tensor("d12", [128, 2 * NCP], bf16, kind="ExternalOutput")
    d3 = nc.dram_tensor("d3", [128, D3_COLS], bf16, kind="ExternalOutput")
    out_bor = nc.dram_tensor("out_bor", [O * 9, BOR_COLS], bf16, kind="ExternalOutput")

    MSPAN_BLKS = 2 * SPAN_BLKS          # 8 blocks per megaspan
    MSPAN_N = MSPAN_BLKS * TILE_N       # 4096 rhs cols
    T2 = 2 * TILE_N                     # 1024: one block's d12 slot

    with tile.TileContext(nc) as tc:
        with (
            tc.tile_pool(name="const", bufs=1) as constp,
            tc.tile_pool(name="sin", bufs=4) as inp,
            tc.tile_pool(name="sout", bufs=2) as outp,
            tc.tile_pool(name="psA", bufs=3, space=bass.MemorySpace.PSUM) as psA,
            tc.tile_pool(name="psB", bufs=2, space=bass.MemorySpace.PSUM) as psB,
        ):
            # one-time loads on HWDGE (no Q7 emission cost, land early)
            wsb = constp.tile([KG, 3072], bf16, tag="wcat")
            nc.sync.dma_start(wsb[:], wcat[:])
            borsb = constp.tile([KG, 4 * BOR_COLS], bf16, tag="bor4")
            nc.scalar.dma_start(borsb[:], bor4[:])
            w1_sb = wsb[0:90, 0:128]
            w2_sb = wsb[0:90, 128:256]
            w3_sb = [wsb[0:90, 256 + a * 128:256 + (a + 1) * 128]
                     for a in range(4)]
            bstA = constp.tile([72, BOR_COLS], bf16, tag="bstA")
            bstB = constp.tile([72, BOR_COLS], bf16, tag="bstB")

            cp = 0  # evac engine alternation counter

            def load_span(b0, nb, e1, e2):
                c0 = b0 * TILE_N
                ncols = nb * TILE_N
                rhs = inp.tile([90, MSPAN_N], bf16, tag="rhs")
                e1.dma_start(
                    rhs[0:36, :ncols],
                    bass.AP(xin18, c0, [[1, 2], [XCOLS2, 18], [1, ncols]]),
                )
                e2.dma_start(
                    rhs[36:90, :ncols],
                    bass.AP(xin18, c0 + S - 1,
                            [[1, 3], [XCOLS2, 18], [1, ncols]]),
                )
                return rhs

            def border():
                nonlocal cp
                for L in range(4):
                    ncol = N_TB if L < 2 else N_LR
                    coff = BOR_OFF[L]
                    for half in range(2):
                        ps = psA.tile([128, T2], f32, tag="p12")
                        for j in range(IGRPS):
                            wc = WB_OFF + (L * IGRPS + j) * 144 + half * 72
                            nc.tensor.matmul(
                                ps[0:72, :ncol],
                                wsb[0:KG, wc:wc + 72],
                                borsb[:, j * BOR_COLS + coff:
                                      j * BOR_COLS + coff + ncol],
                                start=(j == 0),
                                stop=(j == IGRPS - 1),
                            )
                        bst = bstA if half == 0 else bstB
                        if cp % 2 == 0:
                            nc.vector.tensor_copy(bst[:, coff:coff + ncol],
                                                  ps[0:72, :ncol])
                        else:
                            nc.scalar.copy(bst[:, coff:coff + ncol],
                                           ps[0:72, :ncol])
                        cp += 1
                nc.sync.dma_start(out_bor[0:72, :], bstA[:])
                nc.scalar.dma_start(out_bor[72:144, :], bstB[:])

            def megaspan(b0, nb, rhs):
                nonlocal cp
                c0 = b0 * TILE_N
                s12 = outp.tile([128, MSPAN_BLKS * T2], bf16, tag="s12")
                s3 = outp.tile([128, 2 * TILE_N], bf16, tag="s3")
                ngrp = -(-nb // SPAN_BLKS)          # d3 groups this megaspan
                for gg in range(ngrp):
                    p3 = psB.tile([128, TILE_N], f32, tag="p3")
                    na = min(SPAN_BLKS, nb - gg * SPAN_BLKS)
                    # process blocks in pairs to amortize LDWEIGHTS of w1/w2
                    for a0 in range(0, na, 2):
                        npair = min(2, na - a0)
                        ptiles = [psA.tile([128, T2], f32, tag="p12",
                                           name=f"p12_{b0}_{gg}_{a0}_{i}")
                                  for i in range(npair)]
                        rbs = [rhs[:, (gg * SPAN_BLKS + a0 + i) * TILE_N:
                                   (gg * SPAN_BLKS + a0 + i + 1) * TILE_N]
                               for i in range(npair)]
                        for i in range(npair):
                            nc.tensor.matmul(ptiles[i][:, 0:TILE_N], w1_sb,
                                             rbs[i], start=True, stop=True)
                        for i in range(npair):
                            nc.tensor.matmul(ptiles[i][:, TILE_N:T2], w2_sb,
                                             rbs[i], start=True, stop=True)
                        for i in range(npair):
                            a = a0 + i
                            nc.tensor.matmul(p3[:], w3_sb[a], rbs[i],
                                             start=(a == 0), stop=(a == na - 1))
                        for i in range(npair):
                            j0 = (gg * SPAN_BLKS + a0 + i) * T2
                            if cp % 2 == 0:
                                nc.vector.tensor_copy(s12[:, j0:j0 + T2],
                                                      ptiles[i][:])
                            else:
                                nc.scalar.copy(s12[:, j0:j0 + T2], ptiles[i][:])
                            cp += 1
                    if cp % 2 == 0:
                        nc.vector.tensor_copy(
                            s3[:, gg * TILE_N:(gg + 1) * TILE_N], p3[:])
                    else:
                        nc.scalar.copy(
                            s3[:, gg * TILE_N:(gg + 1) * TILE_N], p3[:])
                    cp += 1
                    # store this group's half of s12 as soon as it's evacuated
                    # (all stores on sync: scalar's engine time is needed for
                    # PSUM evacuation copies)
                    h0 = gg * SPAN_BLKS * T2
                    hn = na * T2
                    nc.sync.dma_start(d12[:, c0 * 2 + h0:c0 * 2 + h0 + hn],
                                      s12[:, h0:h0 + hn])
                g0 = b0 // SPAN_BLKS
                nc.sync.dma_start(
                    d3[:, g0 * TILE_N:(g0 + ngrp) * TILE_N],
                    s3[:, :ngrp * TILE_N])

            # staggered spans: small first span lands fast (short prologue),
            # small last span drains fast (short tail). Early spans load via
            # HWDGE (fast engines, stores not running yet); later spans via
            # SWDGE (even engine spread, coexists with stores). Border after
            # megaspan 1 (bor4 arrives meanwhile on the scalar ring).
            SPANS = [(0, 4), (4, 8), (12, 8), (20, 8), (28, 5)]
            rhs_t = {}
            rhs_t[0] = load_span(*SPANS[0], nc.sync, nc.scalar)
            rhs_t[1] = load_span(*SPANS[1], nc.sync, nc.scalar)
            rhs_t[2] = load_span(*SPANS[2], nc.gpsimd, nc.gpsimd)
            megaspan(*SPANS[0], rhs_t[0])
            rhs_t[3] = load_span(*SPANS[3], nc.gpsimd, nc.gpsimd)
            megaspan(*SPANS[1], rhs_t[1])
            border()
            rhs_t[4] = load_span(*SPANS[4], nc.gpsimd, nc.gpsimd)
            megaspan(*SPANS[2], rhs_t[2])
            megaspan(*SPANS[3], rhs_t[3])
            megaspan(*SPANS[4], rhs_t[4])

    nc.finalize()
    _CACHE["nc"] = nc
    return nc


# device K-row order: xin18 stacks the two position blocks as rows, so one
# 3-dim AP covers (tap-group x 18 rows): p = t*18 + b*9 + q
XCOLS2 = 17056


def _mk_perm():
    perm = np.empty(90, dtype=int)
    for p in range(36):
        t, rem = divmod(p, 18)
        b, q = divmod(rem, 9)
        perm[p] = b * 45 + t * 9 + q
    for p in range(36, 90):
        t, rem = divmod(p - 36, 18)
        b, q = divmod(rem, 9)
        perm[p] = b * 45 + (t + 2) * 9 + q
    return perm


_PERM = _mk_perm()


def _host_prep(x, g):
    w1, w2, w3, w_bor = _weights(g)
    wcat = np.zeros((KG, 3072), np.float32)
    wcat[0:90, 0:128] = w1[_PERM]
    wcat[0:90, 128:256] = w2[_PERM]
    wcat[0:90, 256:768] = w3[_PERM]
    wcat[:, WB_OFF:] = w_bor
    pieces = {"w1": w1, "w2": w2, "w3": w3, "w_bor": w_bor}
    wcat = wcat.astype(BF16)
    dev_maps, sim_maps = [], []
    for c in range(NCORES):
        xin9 = _prep_xin_int(x, c)
        xbor = _prep_xin_bor(x, c)
        bor4 = np.ascontiguousarray(
            xbor.reshape(4, KG, BOR_COLS).transpose(1, 0, 2).reshape(KG, 4 * BOR_COLS)
        )
        xin18 = np.empty((18, XCOLS2), np.float32)
        xin18[0:9] = xin9[:, :XCOLS2]
        xin18[9:18] = xin9[:, HALF:HALF + XCOLS2]
        dev_maps.append({
            "xin18": xin18.astype(BF16),
            "bor4": bor4.astype(BF16),
            "wcat": wcat,
        })
        sim_maps.append({"xin9": xin9.astype(BF16), "xin_bor": xbor.astype(BF16),
                         **pieces})
    return dev_maps, sim_maps


def kernel(x, g):
    x = np.ascontiguousarray(np.asarray(x, dtype=np.float32))
    g = np.asarray(g, dtype=np.float32)
    dev_maps, sim_maps = _host_prep(x, g)
    import os
    if os.environ.get("KERNEL_SIM"):
        results = [_sim_core(m, fp=BF16) for m in sim_maps]
        return _assemble(results)
    nc = _build_module()
    from concourse.bass_utils import run_bass_kernel_spmd
    res = run_bass_kernel_spmd(nc, dev_maps, list(range(NCORES)))
    _CACHE["last_result"] = res
    return _assemble(res.results)


# revision 31
# speedup vs baseline: 1.3218x; 1.3218x over previous
"""CayleyConv forward on 8 Trainium2 NeuronCores (Bass/Tile), bf16 I/O.

Problem: x [16,16,128,128,3,3] f32, g [16,16,8,3] f32
         -> out [16,16,130,130,3,3] f32.

Math (faithful to the reference's sequential-overwrite semantics):
  - Interior (rows/cols 1..128) is a 5-tap stencil on xl = x[:, -1] only:
    a [45, 144] contraction per padded position (taps x 9ch -> 16ch x 9).
  - The 1-pixel border keeps full accumulation over all input channels:
    per border line a K=432 contraction.

Sharding: data parallel over batch, 2 images/core. Weights built on host.

Device-side design (v2):
  - All device I/O in bf16 (tolerance 2e-2; bf16 error ~4e-3): halves HBM
    traffic vs f32 -> ~14.4 MB/core, ~40us at the 358 GB/s HBM/NC limit.
  - Position-pair stacking: rhs [90, n] holds the 45 tap-rows for position
    column j (rows 0:45) and j+HALF (rows 45:90). Three matmul streams with
    block-diagonal weights produce 288 output rows per column-pair in 3
    matmuls (M=128/128/32) -> 1.5 streamed cols/position instead of 2.
  - Stream 3 (M=32) accumulates 4 blocks into one [128,512] PSUM bank via
    zero-padded weight variants (block a lands on partitions 32a..32a+31),
    so PSUM evacuation is one [128,512] copy per span.
  - PSUM evacuation (the FD-cost-bound stage: cost = free-dim elems per
    instruction regardless of partitions) alternates Vector/Scalar engines,
    always 128 partitions x 512 cols, casting f32->bf16 during the copy.
  - Stores are [128, n] bf16 tiles -> even descriptor spread over SDMA
    engines; interior loads are DMA-replicated tap reads (grouped taps
    {0,+1} and {+S-1,+S,+S+1} as 2 APs per position-block).
"""

import numpy as np
import ml_dtypes

BF16 = ml_dtypes.bfloat16

# ---------------- problem constants (hardcoded per contract) ----------------
B, CIN, O, H, W = 16, 16, 16, 128, 128
NCORES = 8
BLOC = B // NCORES          # 2 images per core
S = W + 1                   # padded row stride (zero pad col at w=W)
PH = H + 1                  # padded rows per image (zero pad row at h=H)
IMG = PH * S                # 16641 padded positions per image
NPOS = BLOC * IMG           # 33282
HALF = NPOS // 2            # 16641 positions per stacked block
TILE_N = 512
NBLK = 33                   # ceil(HALF / TILE_N) pair-column blocks
NCP = NBLK * TILE_N         # 16896 padded pair-columns
XIN_COLS = 2 * NCP + 132    # 33924: tail pad so +HALF+130 reads stay in-bounds
SPAN_BLKS = 4               # blocks per span (store granularity)
NSPAN = -(-NBLK // SPAN_BLKS)   # 9 spans
D3_COLS = NSPAN * TILE_N    # 4608
TAP_OFFS = (0, 1, S - 1, S, S + 1)
N_TB = BLOC * (W + 2)       # 260 (top/bottom line positions)
N_LR = BLOC * H             # 256 (left/right line positions)
BOR_OFF = (0, N_TB, 2 * N_TB, 2 * N_TB + N_LR)
BOR_COLS = 2 * N_TB + 2 * N_LR  # 1032
KBOR = CIN * 27             # 432 contraction rows for border
IGRPS = 4
KG = KBOR // IGRPS          # 108
EPS = 1e-7

_CACHE = {}


# ---------------- host-side math (tiny) ----------------
def _build_G(g):
    # g: [O, I, 8, 3] f32 -> Cayley matrices G [O, I, 3, 3, 3, 3]
    idx = np.array([[0, 1, 2], [3, 4, 5], [6, 7, 4]])
    gk = g[:, :, idx, :]
    a_, b_, c_ = gk[..., 0], gk[..., 1], gk[..., 2]
    z = np.zeros_like(a_)
    A = np.stack([
        np.stack([z, a_, b_], -1),
        np.stack([-a_, z, c_], -1),
        np.stack([-b_, -c_, z], -1),
    ], -2)
    I3 = np.eye(3, dtype=g.dtype)
    Xm = A.copy(); Xm[:, :, 1, 1] = I3
    Idm = (I3 - A); Idm[:, :, 1, 1] = I3
    bm = Idm
    b00 = bm[..., 0, 0]; b01 = bm[..., 0, 1]; b02 = bm[..., 0, 2]
    b10 = bm[..., 1, 0]; b11 = bm[..., 1, 1]; b12 = bm[..., 1, 2]
    b20 = bm[..., 2, 0]; b21 = bm[..., 2, 1]; b22 = bm[..., 2, 2]
    det = (b00 * (b11 * b22 - b12 * b21)
           - b01 * (b10 * b22 - b12 * b20)
           + b02 * (b10 * b21 - b11 * b20))
    cof = np.stack([
        np.stack([b11 * b22 - b12 * b21, b02 * b21 - b01 * b22, b01 * b12 - b02 * b11], -1),
        np.stack([b12 * b20 - b10 * b22, b00 * b22 - b02 * b20, b02 * b10 - b00 * b12], -1),
        np.stack([b10 * b21 - b11 * b20, b01 * b20 - b00 * b21, b00 * b11 - b01 * b10], -1),
    ], -2)
    inv = cof / (det + EPS)[..., None, None]
    return (inv @ (I3 + Xm)).astype(np.float32)


def _weights(g):
    """Returns (w1 [90,128], w2 [90,128], w3 [90,512], w_bor [108,2304]) f32."""
    # (packed into one [108, 3072] DRAM tensor for a single low-descriptor DMA)
    G = _build_G(g)                                     # [O, I, 3, 3, 3, 3]
    K9 = np.einsum('oiabpq,oiabts->oiabptqs', G, G).reshape(O, CIN, 3, 3, 9, 9)
    K9 = K9.astype(np.float32)

    # interior: taps in DMA-offset order [0, +1, +S-1, +S, +S+1]
    K_int = np.empty((5, O, 9, 9), np.float32)
    K_int[0] = np.eye(9, dtype=np.float32)
    K_int[1] = K9[:, CIN - 1, 1, 0]
    K_int[2] = K9[:, CIN - 1, 0, 2]
    K_int[3] = K9[:, CIN - 1, 0, 1]
    K_int[4] = K9[:, CIN - 1, 0, 0]
    # w_int[t*9+q, o*9+p] = K_int[t, o, p, q]
    w_int = np.ascontiguousarray(K_int.transpose(0, 3, 1, 2).reshape(45, O * 9))

    # block-diagonal packing over the two stacked position blocks:
    #  stream 1 [90,128]: block1 output rows 0..127
    #  stream 2 [90,128]: cols 0:16 = block1 rows 128..143 (top half),
    #                     cols 16:128 = block2 rows 0..111 (bottom half)
    #  stream 3: 4 zero-padded [90,128] variants; variant a holds block2
    #            rows 112..143 at output partitions 32a..32a+31 (accumulated
    #            over the 4 blocks of a span in one PSUM bank)
    w1 = np.zeros((90, 128), np.float32)
    w1[0:45] = w_int[:, 0:128]
    w2 = np.zeros((90, 128), np.float32)
    w2[0:45, 0:16] = w_int[:, 128:144]
    w2[45:90, 16:128] = w_int[:, 0:112]
    w3 = np.zeros((90, 4 * 128), np.float32)
    for a in range(4):
        w3[45:90, a * 128 + 32 * a: a * 128 + 32 * a + 32] = w_int[:, 112:144]

    # border lines: top (a=0, taps b), bottom (a=2, taps b),
    #               left (b=0, taps a), right (b=2, taps a)
    w_bor = np.empty((KG, 4 * IGRPS * O * 9), np.float32)
    sels = [K9[:, :, 0, :], K9[:, :, 2, :], K9[:, :, :, 0], K9[:, :, :, 2]]
    for L, KL in enumerate(sels):                       # KL [O, I, 3, 9p, 9q]
        WL = KL.transpose(1, 2, 4, 0, 3).reshape(KBOR, O * 9)  # rows (i,t,q)
        for j in range(IGRPS):
            w_bor[:, (L * IGRPS + j) * 144:(L * IGRPS + j + 1) * 144] = \
                WL[j * KG:(j + 1) * KG]
    return w1, w2, w3, w_bor


def _prep_xin_int(x, c):
    xsl = x[BLOC * c:BLOC * c + BLOC, CIN - 1]          # [2, H, W, 3, 3]
    xl9 = xsl.reshape(BLOC, H, W, 9).transpose(3, 0, 1, 2)
    tmp = np.zeros((9, BLOC, PH, S), np.float32)
    tmp[:, :, :H, :W] = xl9
    out = np.zeros((9, XIN_COLS), np.float32)
    out[:, :NPOS] = tmp.reshape(9, NPOS)
    return out


def _prep_xin_bor(x, c):
    x9 = x[BLOC * c:BLOC * c + BLOC].reshape(BLOC, CIN, H, W, 9)
    bor = np.zeros((KBOR, BOR_COLS), np.float32)
    for li, h_in in ((0, 0), (1, H - 1)):               # top, bottom
        rT = x9[:, :, h_in].transpose(1, 3, 0, 2)       # [I, 9, 2, W]
        blk = np.zeros((CIN, 3, 9, BLOC, W + 2), np.float32)
        for b in range(3):
            blk[:, b, :, :, b:b + W] = rT
        bor[:, BOR_OFF[li]:BOR_OFF[li] + N_TB] = blk.reshape(KBOR, N_TB)
    for li, w_in in ((2, 0), (3, W - 1)):               # left, right
        cT = x9[:, :, :, w_in].transpose(1, 3, 0, 2)    # [I, 9, 2, H]
        blk = np.zeros((CIN, 3, 9, BLOC, H), np.float32)
        for a in range(3):
            h_lo = max(0, 1 - a)
            h_hi = min(H - 1, H - a)
            blk[:, a, :, :, h_lo + a - 1:h_hi + a] = cT[:, :, :, h_lo:h_hi + 1]
        bor[:, BOR_OFF[li]:BOR_OFF[li] + N_LR] = blk.reshape(KBOR, N_LR)
    return bor


def _decode_full(res):
    """Rebuild full [144, NPOS] f32 from d12/d3 device outputs."""
    # d12 [128, 2*NCP]: block b cols [1024b,1024b+512) = stream1 (d1),
    # [1024b+512,1024b+1024) = stream2 (d2)
    d12 = np.asarray(res["d12"]).astype(np.float32).reshape(128, NBLK, 2, TILE_N)
    d1 = d12[:, :, 0, :].reshape(128, NCP)
    d2 = d12[:, :, 1, :].reshape(128, NCP)
    d3 = np.asarray(res["d3"]).astype(np.float32)       # [128, D3_COLS]
    full = np.empty((144, NPOS), np.float32)
    full[0:128, 0:HALF] = d1[:, :HALF]
    full[128:144, 0:HALF] = d2[0:16, :HALF]
    full[0:112, HALF:] = d2[16:128, :HALF]
    # d3[32a+r, g*512+u] = block2 row 112+r at pair-col (4g+a)*512+u
    d3r = d3.reshape(4, 32, NSPAN, TILE_N)              # [a, r, g, u]
    blk2 = d3r.transpose(1, 2, 0, 3).reshape(32, NSPAN * 4 * TILE_N)
    full[112:144, HALF:] = blk2[:, :HALF]
    return full


def _assemble(results):
    out = np.empty((B, O, H + 2, W + 2, 3, 3), np.float32)
    for c in range(NCORES):
        ms = slice(BLOC * c, BLOC * c + BLOC)
        full = _decode_full(results[c])
        t = full.reshape(O, 9, BLOC, PH, S)[:, :, :, :H, :W]
        out[ms, :, 1:H + 1, 1:W + 1] = \
            t.transpose(2, 0, 3, 4, 1).reshape(BLOC, O, H, W, 3, 3)
        ob = np.asarray(results[c]["out_bor"]).astype(np.float32)
        ob = ob.reshape(O, 9, BOR_COLS)
        top = ob[:, :, 0:N_TB].reshape(O, 9, BLOC, W + 2)
        out[ms, :, 0, :] = top.transpose(2, 0, 3, 1).reshape(BLOC, O, W + 2, 3, 3)
        bot = ob[:, :, N_TB:2 * N_TB].reshape(O, 9, BLOC, W + 2)
        out[ms, :, H + 1, :] = bot.transpose(2, 0, 3, 1).reshape(BLOC, O, W + 2, 3, 3)
        lef = ob[:, :, BOR_OFF[2]:BOR_OFF[2] + N_LR].reshape(O, 9, BLOC, H)
        out[ms, :, 1:H + 1, 0] = lef.transpose(2, 0, 3, 1).reshape(BLOC, O, H, 3, 3)
        rig = ob[:, :, BOR_OFF[3]:BOR_OFF[3] + N_LR].reshape(O, 9, BLOC, H)
        out[ms, :, 1:H + 1, W + 1] = rig.transpose(2, 0, 3, 1).reshape(BLOC, O, H, 3, 3)
    return out


# ---------------- numpy simulation of the device program ----------------
def _sim_core(inm, fp=np.float32):
    """Simulate the device kernel for one core's input map (for validation)."""
    xin9 = np.asarray(inm["xin9"]).astype(np.float32)
    w1 = np.asarray(inm["w1"]).astype(np.float32)
    w2 = np.asarray(inm["w2"]).astype(np.float32)
    w3 = np.asarray(inm["w3"]).astype(np.float32)
    w_bor = np.asarray(inm["w_bor"]).astype(np.float32)
    xin_bor = np.asarray(inm["xin_bor"]).astype(np.float32)

    def taps(c0, n):
        r = np.empty((45, n), np.float32)
        for t, off in enumerate(TAP_OFFS):
            r[t * 9:(t + 1) * 9] = xin9[:, c0 + off:c0 + off + n]
        return r

    d12 = np.zeros((128, NBLK, 2, TILE_N), fp)
    d3 = np.zeros((128, D3_COLS), fp)
    for b0 in range(0, NBLK, SPAN_BLKS):
        nb = min(SPAN_BLKS, NBLK - b0)
        c0 = b0 * TILE_N
        ncols = nb * TILE_N
        rhs = np.concatenate([taps(c0, ncols), taps(c0 + HALF, ncols)], 0)
        p3 = np.zeros((128, TILE_N), np.float32)
        for a in range(nb):
            j0 = a * TILE_N
            rb = rhs[:, j0:j0 + TILE_N]
            d12[:, b0 + a, 0] = (w1.T @ rb).astype(fp)
            d12[:, b0 + a, 1] = (w2.T @ rb).astype(fp)
            p3 += w3[:, a * 128:(a + 1) * 128].T @ rb
        d3[:, (b0 // SPAN_BLKS) * TILE_N:(b0 // SPAN_BLKS + 1) * TILE_N] = \
            p3.astype(fp)
    d12 = d12.reshape(128, 2 * NCP)

    ret = {"d12": d12, "d3": d3}
    out_bor = np.zeros((144, BOR_COLS), fp)
    for L in range(4):
        ncol = N_TB if L < 2 else N_LR
        coff = BOR_OFF[L]
        for half in range(2):
            acc = np.zeros((72, ncol), np.float32)
            for j in range(IGRPS):
                wc = (L * IGRPS + j) * 144 + half * 72
                acc += w_bor[:, wc:wc + 72].T @ \
                    xin_bor[j * KG:(j + 1) * KG, coff:coff + ncol]
            out_bor[half * 72:half * 72 + 72, coff:coff + ncol] = acc.astype(fp)
    ret["out_bor"] = out_bor
    return ret


# ---------------- device kernel ----------------
# wcat layout: [108, 3072] bf16. rows 0:90 cols 0:128 = w1, 128:256 = w2,
# 256:768 = w3 (4 variants); rows 0:108 cols 768:3072 = w_bor [108, 2304].
WB_OFF = 768


def _build_module():
    if "nc" in _CACHE:
        return _CACHE["nc"]
    import concourse.bass as bass
    import concourse.mybir as mybir
    import concourse.tile as tile
    from concourse import bacc

    f32 = mybir.dt.float32
    bf16 = mybir.dt.bfloat16
    nc = bacc.Bacc(None, target_bir_lowering=False)

    xin18 = nc.dram_tensor("xin18", [18, XCOLS2], bf16, kind="ExternalInput")
    bor4 = nc.dram_tensor("bor4", [KG, 4 * BOR_COLS], bf16, kind="ExternalInput")
    wcat = nc.dram_tensor("wcat", [KG, 3072], bf16, kind="ExternalInput")
    d12 = nc.dram_tensor("d12", [128, 2 * NCP], bf16, kind="ExternalOutput")
    d3 = nc.dram_tensor("d3", [128, D3_COLS], bf16, kind="ExternalOutput")
    out_bor = nc.dram_tensor("out_bor", [O * 9, BOR_COLS], bf16, kind="ExternalOutput")

    MSPAN_BLKS = 2 * SPAN_BLKS          # 8 blocks per megaspan
    MSPAN_N = MSPAN_BLKS * TILE_N       # 4096 rhs cols
    T2 = 2 * TILE_N                     # 1024: one block's d12 slot

    with tile.TileContext(nc) as tc:
        with (
            tc.tile_pool(name="const", bufs=1) as constp,
            tc.tile_pool(name="sin", bufs=4) as inp,
            tc.tile_pool(name="sout", bufs=2) as outp,
            tc.tile_pool(name="psA", bufs=3, space=bass.MemorySpace.PSUM) as psA,
            tc.tile_pool(name="psB", bufs=2, space=bass.MemorySpace.PSUM) as psB,
        ):
            # one-time loads on HWDGE (no Q7 emission cost, land early)
            wsb = constp.tile([KG, 3072], bf16, tag="wcat")
            nc.sync.dma_start(wsb[:], wcat[:])
            borsb = constp.tile([KG, 4 * BOR_COLS], bf16, tag="bor4")
            nc.scalar.dma_start(borsb[:], bor4[:])
            w1_sb = wsb[0:90, 0:128]
            w2_sb = wsb[0:90, 128:256]
            w3_sb = [wsb[0:90, 256 + a * 128:256 + (a + 1) * 128]
                     for a in range(4)]
            bstA = constp.tile([72, BOR_COLS], bf16, tag="bstA")
            bstB = constp.tile([72, BOR_COLS], bf16, tag="bstB")

            cp = 0  # evac engine alternation counter

            def load_span(b0, nb):
                # v5 structure: 4 SWDGE loads (2 tap-groups x 2 blocks), rows
                # in unpermuted order (b*45 + t*9 + q)
                c0 = b0 * TILE_N
                ncols = nb * TILE_N
                B2 = 9 * XCOLS2
                rhs = inp.tile([90, MSPAN_N], bf16, tag="rhs")
                nc.gpsimd.dma_start(
                    rhs[0:18, :ncols],
                    bass.AP(xin18, c0, [[1, 2], [XCOLS2, 9], [1, ncols]]),
                )
                nc.gpsimd.dma_start(
                    rhs[18:45, :ncols],
                    bass.AP(xin18, c0 + S - 1, [[1, 3], [XCOLS2, 9], [1, ncols]]),
                )
                nc.gpsimd.dma_start(
                    rhs[45:63, :ncols],
                    bass.AP(xin18, B2 + c0, [[1, 2], [XCOLS2, 9], [1, ncols]]),
                )
                nc.gpsimd.dma_start(
                    rhs[63:90, :ncols],
                    bass.AP(xin18, B2 + c0 + S - 1,
                            [[1, 3], [XCOLS2, 9], [1, ncols]]),
                )
                return rhs

            def border():
                nonlocal cp
                for L in range(4):
                    ncol = N_TB if L < 2 else N_LR
                    coff = BOR_OFF[L]
                    for half in range(2):
                        ps = psA.tile([128, T2], f32, tag="p12")
                        for j in range(IGRPS):
                            wc = WB_OFF + (L * IGRPS + j) * 144 + half * 72
                            nc.tensor.matmul(
                                ps[0:72, :ncol],
                                wsb[0:KG, wc:wc + 72],
                                borsb[:, j * BOR_COLS + coff:
                                      j * BOR_COLS + coff + ncol],
                                start=(j == 0),
                                stop=(j == IGRPS - 1),
                            )
                        bst = bstA if half == 0 else bstB
                        if cp % 2 == 0:
                            nc.vector.tensor_copy(bst[:, coff:coff + ncol],
                                                  ps[0:72, :ncol])
                        else:
                            nc.scalar.copy(bst[:, coff:coff + ncol],
                                           ps[0:72, :ncol])
                        cp += 1
                nc.sync.dma_start(out_bor[0:72, :], bstA[:])
                nc.scalar.dma_start(out_bor[72:144, :], bstB[:])

            def megaspan(b0, nb, rhs):
                nonlocal cp
                c0 = b0 * TILE_N
                s12 = outp.tile([128, MSPAN_BLKS * T2], bf16, tag="s12")
                s3 = outp.tile([128, 2 * TILE_N], bf16, tag="s3")
                ngrp = -(-nb // SPAN_BLKS)          # d3 groups this megaspan
                for gg in range(ngrp):
                    p3 = psB.tile([128, TILE_N], f32, tag="p3")
                    na = min(SPAN_BLKS, nb - gg * SPAN_BLKS)
                    # process blocks in pairs to amortize LDWEIGHTS of w1/w2
                    for a0 in range(0, na, 2):
                        npair = min(2, na - a0)
                        ptiles = [psA.tile([128, T2], f32, tag="p12",
                                           name=f"p12_{b0}_{gg}_{a0}_{i}")
                                  for i in range(npair)]
                        rbs = [rhs[:, (gg * SPAN_BLKS + a0 + i) * TILE_N:
                                   (gg * SPAN_BLKS + a0 + i + 1) * TILE_N]
                               for i in range(npair)]
                        for i in range(npair):
                            nc.tensor.matmul(ptiles[i][:, 0:TILE_N], w1_sb,
                                             rbs[i], start=True, stop=True)
                        for i in range(npair):
                            nc.tensor.matmul(ptiles[i][:, TILE_N:T2], w2_sb,
                                             rbs[i], start=True, stop=True)
                        for i in range(npair):
                            a = a0 + i
                            nc.tensor.matmul(p3[:], w3_sb[a], rbs[i],
                                             start=(a == 0), stop=(a == na - 1))
                        for i in range(npair):
                            j0 = (gg * SPAN_BLKS + a0 + i) * T2
                            if cp % 2 == 0:
                                nc.vector.tensor_copy(s12[:, j0:j0 + T2],
                                                      ptiles[i][:])
                            else:
                                nc.scalar.copy(s12[:, j0:j0 + T2], ptiles[i][:])
                            cp += 1
                    if cp % 2 == 0:
                        nc.vector.tensor_copy(
                            s3[:, gg * TILE_N:(gg + 1) * TILE_N], p3[:])
                    else:
                        nc.scalar.copy(
                            s3[:, gg * TILE_N:(gg + 1) * TILE_N], p3[:])
                    cp += 1
                    # store this group's half of s12 as soon as it's evacuated
                    h0 = gg * SPAN_BLKS * T2
                    hn = na * T2
                    heng = nc.sync if gg % 2 == 0 else nc.scalar
                    heng.dma_start(d12[:, c0 * 2 + h0:c0 * 2 + h0 + hn],
                                   s12[:, h0:h0 + hn])
                g0 = b0 // SPAN_BLKS
                nc.sync.dma_start(
                    d3[:, g0 * TILE_N:(g0 + ngrp) * TILE_N],
                    s3[:, :ngrp * TILE_N])

            # staggered spans: small first span lands fast (short prologue),
            # small last span drains fast (short tail). Early spans load via
            # HWDGE (fast engines, stores not running yet); later spans via
            # SWDGE (even engine spread, coexists with stores). Border after
            # megaspan 1 (bor4 arrives meanwhile on the scalar ring).
            SPANS = [(0, 4), (4, 8), (12, 8), (20, 8), (28, 5)]
            rhs_t = {}
            for i in range(3):
                rhs_t[i] = load_span(*SPANS[i])
            megaspan(*SPANS[0], rhs_t[0])
            border()
            rhs_t[3] = load_span(*SPANS[3])
            megaspan(*SPANS[1], rhs_t[1])
            rhs_t[4] = load_span(*SPANS[4])
            megaspan(*SPANS[2], rhs_t[2])
            megaspan(*SPANS[3], rhs_t[3])
            megaspan(*SPANS[4], rhs_t[4])

    nc.finalize()
    _CACHE["nc"] = nc
    return nc


# device K-row order: xin18 stacks the two position blocks as rows, so one
# 3-dim AP covers (tap-group x 18 rows): p = t*18 + b*9 + q
XCOLS2 = 17056


def _mk_perm():
    perm = np.empty(90, dtype=int)
    for p in range(36):
        t, rem = divmod(p, 18)
        b, q = divmod(rem, 9)
        perm[p] = b * 45 + t * 9 + q
    for p in range(36, 90):
        t, rem = divmod(p - 36, 18)
        b, q = divmod(rem, 9)
        perm[p] = b * 45 + (t + 2) * 9 + q
    return perm


_PERM = _mk_perm()


def _host_prep(x, g):
    w1, w2, w3, w_bor = _weights(g)
    wcat = np.zeros((KG, 3072), np.float32)
    wcat[0:90, 0:128] = w1
    wcat[0:90, 128:256] = w2
    wcat[0:90, 256:768] = w3
    wcat[:, WB_OFF:] = w_bor
    pieces = {"w1": w1, "w2": w2, "w3": w3, "w_bor": w_bor}
    wcat = wcat.astype(BF16)
    dev_maps, sim_maps = [], []
    for c in range(NCORES):
        xin9 = _prep_xin_int(x, c)
        xbor = _prep_xin_bor(x, c)
        bor4 = np.ascontiguousarray(
            xbor.reshape(4, KG, BOR_COLS).transpose(1, 0, 2).reshape(KG, 4 * BOR_COLS)
        )
        xin18 = np.empty((18, XCOLS2), np.float32)
        xin18[0:9] = xin9[:, :XCOLS2]
        xin18[9:18] = xin9[:, HALF:HALF + XCOLS2]
        dev_maps.append({
            "xin18": xin18.astype(BF16),
            "bor4": bor4.astype(BF16),
            "wcat": wcat,
        })
        sim_maps.append({"xin9": xin9.astype(BF16), "xin_bor": xbor.astype(BF16),
                         **pieces})
    return dev_maps, sim_maps


def kernel(x, g):
    x = np.ascontiguousarray(np.asarray(x, dtype=np.float32))
    g = np.asarray(g, dtype=np.float32)
    dev_maps, sim_maps = _host_prep(x, g)
    import os
    if os.environ.get("KERNEL_SIM"):
        results = [_sim_core(m, fp=BF16) for m in sim_maps]
        return _assemble(results)
    nc = _build_module()
    from concourse.bass_utils import run_bass_kernel_spmd
    res = run_bass_kernel_spmd(nc, dev_maps, list(range(NCORES)))
    _CACHE["last_result"] = res
    return _assemble(res.results)
